# revision 6
# baseline (speedup 1.0000x reference)
"""Trainium2 Bass kernel for nn_CrossAttnBlock (sparse_attention, memory-bound).

Math note: in the reference, the attention logits are broadcast along the
*key* axis before the softmax, so the softmax runs over a constant vector
and is exactly uniform (1/(H*W)).  The attention output therefore collapses
to v broadcast over space, and the whole block reduces to

    out[b,c,h,w] = x[b,c,h,w] + (w3 @ (w2 @ context[b] + b2) + b3)[c]

GroupNorm / q / k are dead code.  The kernel streams x (memory-bound) and
computes the two tiny matvecs on the tensor engine.

Sharding: pure data parallel over batch (B=8 -> 1 batch element per core);
params replicated on every core.

All matvec constants are packed into one DRAM tensor so they arrive via a
single DMA: walrus allows only one sync-wait on a Matmult (it rides the
LoadWeights slot), so the first matmul may depend on at most one DMA queue.
"""

import numpy as np

import concourse.bass as bass
import concourse.bacc as bacc
import concourse.tile as tile
from concourse import mybir
from concourse.bass_utils import run_bass_kernel_spmd

N_CORES = 8
B, C, H, W, CC = 8, 256, 48, 48, 512
S = H * W              # 2304 spatial positions
P = 128                # SBUF partitions
CI = C // P            # 2 channel chunks
KJ = CC // P           # 4 contraction chunks for w2 (k = p*KJ + j)
FC = 768               # free-dim chunk of the x stream
NF = S // FC           # 3 chunks per channel tile

# packed-constant column offsets
OFF_W2 = 0                  # [P, KJ*C]   (p, j*C+m) = w2[m, p*KJ+j]
OFF_CTX = OFF_W2 + KJ * C   # [P, KJ]     (p, j)     = context[p*KJ+j]
OFF_W3 = OFF_CTX + KJ       # [P, CI*C]   (p, mi*C+o) = w3[o, mi*P+p]
OFF_B2 = OFF_W3 + CI * C    # [P, CI]     (p, mi)    = b2[mi*P+p]
OFF_B3 = OFF_B2 + CI        # [P, CI]     (p, oi)    = b3[oi*P+p]
PACK_COLS = OFF_B3 + CI     # 1544

_DT = mybir.dt.float32


def build_nc(loop_r: int = 1) -> bass.Bass:
    # Bacc (not raw Bass): its finalize pipeline runs generate_event_semaphores,
    # which splits multi-waits — TRN2 allows at most 1 sync wait per instruction.
    nc = bacc.Bacc()

    x_d = nc.dram_tensor("x", [CI, P, S], _DT, kind="ExternalInput")
    wp_d = nc.dram_tensor("wpack", [P, PACK_COLS], _DT, kind="ExternalInput")
    out_d = nc.dram_tensor("out", [CI, P, S], _DT, kind="ExternalOutput")

    with tile.TileContext(nc) as tc:
        with (
            tc.tile_pool(name="consts", bufs=2) as consts,
            tc.tile_pool(name="small", bufs=2) as small,
            tc.tile_pool(name="psum", bufs=2, space="PSUM") as psum,
            tc.tile_pool(name="stream", bufs=4) as stream,
        ):
            # loop_r > 1 repeats the whole body back-to-back inside one NEFF;
            # used only for slope-based wall-clock timing (see bench.py).
            for _ in range(loop_r):
                wp = consts.tile([P, PACK_COLS], _DT, tag="wp")
                nc.sync.dma_start(out=wp, in_=wp_d[:])

                # v[mi*P+p] = sum_k w2[m,k] * context[k], k ordered p*KJ + j
                psum_v = psum.tile([P, CI], _DT, tag="pv")
                v_sb = small.tile([P, CI], _DT, tag="v")
                for mi in range(CI):
                    for j in range(KJ):
                        nc.tensor.matmul(
                            psum_v[:, mi : mi + 1],
                            lhsT=wp[
                                :, OFF_W2 + j * C + mi * P : OFF_W2 + j * C + (mi + 1) * P
                            ],
                            rhs=wp[:, OFF_CTX + j : OFF_CTX + j + 1],
                            start=(j == 0),
                            stop=(j == KJ - 1),
                        )
                nc.vector.tensor_add(v_sb, psum_v, wp[:, OFF_B2 : OFF_B2 + CI])

                # proj[oi*P+p] = sum_m w3[o,m] * v[m], m ordered mi*P + p
                psum_p = psum.tile([P, CI], _DT, tag="pp")
                proj_sb = small.tile([P, CI], _DT, tag="proj")
                for oi in range(CI):
                    for mi in range(CI):
                        nc.tensor.matmul(
                            psum_p[:, oi : oi + 1],
                            lhsT=wp[
                                :,
                                OFF_W3 + mi * C + oi * P : OFF_W3 + mi * C + (oi + 1) * P,
                            ],
                            rhs=v_sb[:, mi : mi + 1],
                            start=(mi == 0),
                            stop=(mi == CI - 1),
                        )
                nc.vector.tensor_add(proj_sb, psum_p, wp[:, OFF_B3 : OFF_B3 + CI])

                # stream x: out = x + proj broadcast along the free dim
                u = 0
                for ci in range(CI):
                    for fj in range(NF):
                        t = stream.tile([P, FC], _DT, tag="t")
                        sl = bass.ts(fj, FC)
                        nc.sync.dma_start(out=t, in_=x_d[ci, :, sl])
                        if u % 2 == 0:
                            nc.vector.tensor_scalar_add(t, t, proj_sb[:, ci : ci + 1])
                        else:
                            nc.scalar.activation(
                                t,
                                t,
                                func=mybir.ActivationFunctionType.Identity,
                                bias=proj_sb[:, ci : ci + 1],
                                scale=1.0,
                            )
                        nc.sync.dma_start(out=out_d[ci, :, sl], in_=t)
                        u += 1

    nc.finalize()
    return nc


def _prep_in_maps(inputs: dict) -> list[dict]:
    f32 = lambda a: np.ascontiguousarray(np.asarray(a), dtype=np.float32)
    x = f32(inputs["x"])                    # [B, C, H, W]
    context = f32(inputs["context"])        # [B, CC]
    w2 = f32(inputs["w2"])                  # [C, CC]
    b2 = f32(inputs["b2"])                  # [C]
    w3 = f32(inputs["w3"])                  # [C, C]
    b3 = f32(inputs["b3"])                  # [C]

    base = np.empty((P, PACK_COLS), dtype=np.float32)
    base[:, OFF_W2 : OFF_W2 + KJ * C] = w2.T.reshape(P, KJ * C)
    base[:, OFF_W3 : OFF_W3 + CI * C] = (
        w3.T.reshape(CI, P, C).transpose(1, 0, 2).reshape(P, CI * C)
    )
    base[:, OFF_B2 : OFF_B2 + CI] = b2.reshape(CI, P).T
    base[:, OFF_B3 : OFF_B3 + CI] = b3.reshape(CI, P).T

    in_maps = []
    for b in range(N_CORES):
        wpack = base.copy()
        wpack[:, OFF_CTX : OFF_CTX + KJ] = context[b].reshape(P, KJ)
        in_maps.append({"x": x[b].reshape(CI, P, S), "wpack": wpack})
    return in_maps


def run(inputs: dict, trace: bool = False):
    """Build+run on 8 cores; returns (full_output, BassKernelResults)."""
    nc = build_nc()
    in_maps = _prep_in_maps(inputs)
    res = run_bass_kernel_spmd(nc, in_maps, list(range(N_CORES)), trace=trace)
    out = np.stack(
        [res.results[b]["out"].reshape(C, H, W) for b in range(N_CORES)], axis=0
    )
    return out.astype(np.float32, copy=False), res


def kernel(**inputs: np.ndarray) -> np.ndarray:
    out, _ = run(inputs, trace=False)
    return out


# revision 11
# speedup vs baseline: 3.7600x; 3.7600x over previous
"""Trainium2 Bass kernel for nn_CrossAttnBlock (sparse_attention, memory-bound).

Math note: in the reference, the attention logits are broadcast along the
*key* axis before the softmax, so the softmax runs over a constant vector
and is exactly uniform (1/(H*W)).  The attention output therefore collapses
to v broadcast over space, and the whole block reduces to

    out[b,c,h,w] = x[b,c,h,w] + (w3 @ (w2 @ context[b] + b2) + b3)[c]

GroupNorm / q / k are dead code.  The kernel streams x (memory-bound) and
computes the two tiny matvecs on the tensor engine.

Sharding: pure data parallel over batch (B=8 -> 1 batch element per core);
params replicated on every core.

All matvec constants are packed into one DRAM tensor so they arrive via a
single DMA: walrus allows only one sync-wait on a Matmult (it rides the
LoadWeights slot), so the first matmul may depend on at most one DMA queue.
"""

import numpy as np

import concourse.bass as bass
import concourse.bacc as bacc
import concourse.tile as tile
from concourse import mybir
from concourse.bass_utils import run_bass_kernel_spmd

N_CORES = 8
B, C, H, W, CC = 8, 256, 48, 48, 512
S = H * W              # 2304 spatial positions
P = 128                # SBUF partitions
CI = C // P            # 2 channel chunks
KJ = CC // P           # 4 contraction chunks for w2 (k = p*KJ + j)
FC = 768               # free-dim chunk of the x stream (default)
NF = S // FC           # 3 chunks per channel tile

# packed-constant column offsets
OFF_W2 = 0                  # [P, KJ*C]   (p, j*C+m) = w2[m, p*KJ+j]
OFF_CTX = OFF_W2 + KJ * C   # [P, KJ]     (p, j)     = context[p*KJ+j]
OFF_W3 = OFF_CTX + KJ       # [P, CI*C]   (p, mi*C+o) = w3[o, mi*P+p]
OFF_B2 = OFF_W3 + CI * C    # [P, CI]     (p, mi)    = b2[mi*P+p]
OFF_B3 = OFF_B2 + CI        # [P, CI]     (p, oi)    = b3[oi*P+p]
PACK_COLS = OFF_B3 + CI     # 1544

_DT = mybir.dt.float32


def build_nc(
    loop_r: int = 1,
    fc: int = FC,
    bufs: int = 4,
    dual_engine: bool = True,
) -> bass.Bass:
    # Bacc (not raw Bass): its finalize pipeline runs generate_event_semaphores,
    # which splits multi-waits — TRN2 allows at most 1 sync wait per instruction.
    nc = bacc.Bacc()

    x_d = nc.dram_tensor("x", [CI, P, S], _DT, kind="ExternalInput")
    wp_d = nc.dram_tensor("wpack", [P, PACK_COLS], _DT, kind="ExternalInput")
    out_d = nc.dram_tensor("out", [CI, P, S], _DT, kind="ExternalOutput")

    with tile.TileContext(nc) as tc:
        with (
            tc.tile_pool(name="consts", bufs=2) as consts,
            tc.tile_pool(name="small", bufs=2) as small,
            tc.tile_pool(name="psum", bufs=2, space="PSUM") as psum,
            tc.tile_pool(name="stream", bufs=bufs) as stream,
        ):
            # loop_r > 1 repeats the whole body back-to-back inside one NEFF;
            # used only for slope-based wall-clock timing (see bench.py).
            for _ in range(loop_r):
                wp = consts.tile([P, PACK_COLS], _DT, tag="wp")
                nc.sync.dma_start(out=wp, in_=wp_d[:])

                # v[mi*P+p] = sum_k w2[m,k] * context[k], k ordered p*KJ + j
                psum_v = psum.tile([P, CI], _DT, tag="pv")
                v_sb = small.tile([P, CI], _DT, tag="v")
                for mi in range(CI):
                    for j in range(KJ):
                        nc.tensor.matmul(
                            psum_v[:, mi : mi + 1],
                            lhsT=wp[
                                :, OFF_W2 + j * C + mi * P : OFF_W2 + j * C + (mi + 1) * P
                            ],
                            rhs=wp[:, OFF_CTX + j : OFF_CTX + j + 1],
                            start=(j == 0),
                            stop=(j == KJ - 1),
                        )
                nc.vector.tensor_add(v_sb, psum_v, wp[:, OFF_B2 : OFF_B2 + CI])

                # proj[oi*P+p] = sum_m w3[o,m] * v[m], m ordered mi*P + p
                psum_p = psum.tile([P, CI], _DT, tag="pp")
                proj_sb = small.tile([P, CI], _DT, tag="proj")
                for oi in range(CI):
                    for mi in range(CI):
                        nc.tensor.matmul(
                            psum_p[:, oi : oi + 1],
                            lhsT=wp[
                                :,
                                OFF_W3 + mi * C + oi * P : OFF_W3 + mi * C + (oi + 1) * P,
                            ],
                            rhs=v_sb[:, mi : mi + 1],
                            start=(mi == 0),
                            stop=(mi == CI - 1),
                        )
                nc.vector.tensor_add(proj_sb, psum_p, wp[:, OFF_B3 : OFF_B3 + CI])

                # stream x: out = x + proj broadcast along the free dim
                assert S % fc == 0
                nf = S // fc
                u = 0
                for ci in range(CI):
                    for fj in range(nf):
                        t = stream.tile([P, fc], _DT, tag="t")
                        sl = bass.ts(fj, fc)
                        nc.sync.dma_start(out=t, in_=x_d[ci, :, sl])
                        if dual_engine and u % 2 == 1:
                            nc.scalar.activation(
                                t,
                                t,
                                func=mybir.ActivationFunctionType.Identity,
                                bias=proj_sb[:, ci : ci + 1],
                                scale=1.0,
                            )
                        else:
                            nc.vector.tensor_scalar_add(t, t, proj_sb[:, ci : ci + 1])
                        nc.sync.dma_start(out=out_d[ci, :, sl], in_=t)
                        u += 1

    nc.finalize()
    return nc


def _prep_in_maps(inputs: dict) -> list[dict]:
    f32 = lambda a: np.ascontiguousarray(np.asarray(a), dtype=np.float32)
    x = f32(inputs["x"])                    # [B, C, H, W]
    context = f32(inputs["context"])        # [B, CC]
    w2 = f32(inputs["w2"])                  # [C, CC]
    b2 = f32(inputs["b2"])                  # [C]
    w3 = f32(inputs["w3"])                  # [C, C]
    b3 = f32(inputs["b3"])                  # [C]

    base = np.empty((P, PACK_COLS), dtype=np.float32)
    base[:, OFF_W2 : OFF_W2 + KJ * C] = w2.T.reshape(P, KJ * C)
    base[:, OFF_W3 : OFF_W3 + CI * C] = (
        w3.T.reshape(CI, P, C).transpose(1, 0, 2).reshape(P, CI * C)
    )
    base[:, OFF_B2 : OFF_B2 + CI] = b2.reshape(CI, P).T
    base[:, OFF_B3 : OFF_B3 + CI] = b3.reshape(CI, P).T

    in_maps = []
    for b in range(N_CORES):
        wpack = base.copy()
        wpack[:, OFF_CTX : OFF_CTX + KJ] = context[b].reshape(P, KJ)
        in_maps.append({"x": x[b].reshape(CI, P, S), "wpack": wpack})
    return in_maps


def run(inputs: dict, trace: bool = False):
    """Build+run on 8 cores; returns (full_output, BassKernelResults)."""
    nc = build_nc()
    in_maps = _prep_in_maps(inputs)
    res = run_bass_kernel_spmd(nc, in_maps, list(range(N_CORES)), trace=trace)
    out = np.stack(
        [res.results[b]["out"].reshape(C, H, W) for b in range(N_CORES)], axis=0
    )
    return out.astype(np.float32, copy=False), res


def kernel(**inputs: np.ndarray) -> np.ndarray:
    out, _ = run(inputs, trace=False)
    return out
